# revision 8
# baseline (speedup 1.0000x reference)
"""Trainium2 Bass kernel for grouped multi-head attention.

Problem: B=16, S=7500, H=64; frames T=300, J=25 joint groups, hs=4 heads,
dk=64.  out = MHA(q,k,v) with per-(b,j,h) attention over the 300-frame axis.

Math restructuring (host does LAYOUT + WEIGHT-FOLDING only, no activation
math):
  scores_h = (q Wq_h)(k Wk_h)^T * dk^-0.5 = q A_h k^T,  A_h = Wq_h Wk_h^T * dk^-0.5
  final    = sum_h rowscale(p_h @ v, 1/rowsum_h) @ G_h,  G_h = Wv_h Wo_h
On device, per (b,j,h)  [t on free axis, s on partitions for exp/PV]:
  zT   (64,300)  = A_h^T q^T           (lhsT=A_h, rhs=qT)
  scT  (s,300)   = k zT                (lhsT=kT chunk, rhs=zT slice)
  pT   (s,300)   = exp(scT)            (ACT, PSUM->SBUF, 4 heads per op)
  wT   (65,300)  = [v|1]^T p^T         (lhsT=[v|1] chunk, rhs=pT chunk, accum)
  r    (1,300)   = 1/wT[64]            (DVE reciprocal)
  rb   (64,300)  = bcast(r)            (GPSIMD partition_broadcast)
  wTn  (64,300)  = wT[:64] * rb        (DVE / GPSIMD)
  finT (64,300) += G_h^T wTn           (lhsT=G_h, rhs=wTn, accum over h)
Sharding: batch B over 8 cores (2 per core).  Host pre-transposes q,k to
(b*64+d, j, t) layout, packs v with a ones column; output is returned
transposed and re-laid-out on host.

PSUM budget (8 banks): scores/z slots (128,1200)=3 banks x bufs=2, wt/fin
slots (65,512)=1 bank x bufs=2.  Matmuls into the packed 1200-wide tiles are
split at 512-f32 bank boundaries.
"""

import sys

for p in ("/opt/trn_rl_repo", "/root/.axon_site/_ro/trn_rl_repo"):
    if p not in sys.path:
        sys.path.insert(0, p)

import numpy as np

import concourse.bass as bass
import concourse.bacc as bacc
import concourse.mybir as mybir
import concourse.tile as tile
from concourse.bass_utils import run_bass_kernel_spmd

B, S, H = 16, 7500, 64
T, HS, DK = 300, 4, 64
J = S // T  # 25
NCORES = 8
BPC = B // NCORES  # batches per core = 2
KS = [128, 128, 44]  # s-chunk sizes (sum = 300)
KOFF = [0, 128, 256]
F32 = mybir.dt.float32
BANK = 512  # fp32 per PSUM bank

_PROG_CACHE = {}
_SKIP_BCAST = False  # timing probe only


def _bank_pieces(off, n=T):
    """Split [off, off+n) at BANK boundaries -> list of (start, len)."""
    out = []
    p = off
    while p < off + n:
        end = min(off + n, (p // BANK + 1) * BANK)
        out.append((p, end - p))
        p = end
    return out


def build_program():
    nc = bacc.Bacc(None, target_bir_lowering=False, debug=False)

    qT = nc.dram_tensor("qT", (128, J, T), F32, kind="ExternalInput")
    kT = nc.dram_tensor("kT", (128, J, T), F32, kind="ExternalInput")
    # v1: [b, j, p(128), c(3), d(65)]  (d=64 is the ones column)
    v1 = nc.dram_tensor("v1", (BPC, J, 128, 3, 65), F32, kind="ExternalInput")
    # A duplicated on both partition halves (b0 rows 0-63, b1 rows 64-127)
    Ad = nc.dram_tensor("Ad", (128, HS, DK), F32, kind="ExternalInput")
    Gd = nc.dram_tensor("Gd", (64, HS, DK), F32, kind="ExternalInput")
    outd = nc.dram_tensor("outd", (64, BPC, J, T), F32, kind="ExternalOutput")

    with tile.TileContext(nc) as tc:
        with (
            tc.tile_pool(name="weights", bufs=1) as wpool,
            tc.tile_pool(name="io", bufs=3) as iopool,
            tc.tile_pool(name="work", bufs=2) as workpool,
            tc.tile_pool(name="pt", bufs=8) as ptpool,
            tc.tile_pool(name="psc", bufs=2, space="PSUM") as psc,
            tc.tile_pool(name="pwt", bufs=2, space="PSUM") as pwt,
        ):
            A_sb = wpool.tile([128, HS, DK], F32, tag="A")
            nc.sync.dma_start(A_sb[:], Ad[:])
            G_sb = wpool.tile([64, HS, DK], F32, tag="G")
            nc.sync.dma_start(G_sb[:], Gd[:])

            for j in range(J):
                qT_sb = iopool.tile([128, T], F32, tag="qT")
                nc.sync.dma_start(qT_sb[:], qT[:, j])
                kT_sb = iopool.tile([128, T], F32, tag="kT")
                nc.sync.dma_start(kT_sb[:], kT[:, j])
                v1_sb = [
                    iopool.tile([128, 3, 65], F32, tag=f"v1_{b}", name=f"v1_{b}")
                    for b in range(BPC)
                ]
                for b in range(BPC):
                    nc.sync.dma_start(v1_sb[b][:], v1[b, j].rearrange("p c d -> p (c d)"))

                # ---- z: psum (128, 4*300) packed; z[b,h] rows 64b:64b+64,
                # cols h*300:(h+1)*300 (split at bank boundaries)
                z_ps = psc.tile([128, HS * T], F32, tag="sc", name="z_ps")
                for b in range(BPC):
                    sl = slice(64 * b, 64 * b + 64)
                    for h in range(HS):
                        for p0, ln in _bank_pieces(h * T):
                            nc.tensor.matmul(
                                z_ps[sl, p0 : p0 + ln],
                                A_sb[sl, h, :],
                                qT_sb[sl, p0 - h * T : p0 - h * T + ln],
                                start=True, stop=True,
                            )
                zT_sb = workpool.tile([128, HS * T], F32, tag="zT")
                nc.vector.tensor_copy(zT_sb[:], z_ps[:])

                out_sb = workpool.tile([64, BPC, T], F32, tag="out")
                for b in range(BPC):
                    sl = slice(64 * b, 64 * b + 64)
                    # ---- scores^T (+exp) per s-chunk, 4 heads packed
                    pT_sb = []
                    for c, kc in enumerate(KS):
                        sc_ps = psc.tile([128, HS * T], F32, tag="sc", name="sc_ps")
                        for h in range(HS):
                            for p0, ln in _bank_pieces(h * T):
                                nc.tensor.matmul(
                                    sc_ps[:kc, p0 : p0 + ln],
                                    kT_sb[sl, KOFF[c] : KOFF[c] + kc],
                                    zT_sb[sl, p0 : p0 + ln],
                                    start=True, stop=True,
                                )
                        p_sb = ptpool.tile([128, HS * T], F32, tag="pT")
                        nc.scalar.activation(
                            p_sb[:kc], sc_ps[:kc],
                            mybir.ActivationFunctionType.Exp,
                        )
                        pT_sb.append(p_sb)

                    # ---- per head: PV accum, recip, bcast, norm
                    wTn_sb = workpool.tile([64, HS, T], F32, tag="wTn")
                    for h in range(HS):
                        wt_ps = pwt.tile([65, BANK], F32, tag="wt", name="wt_ps")
                        for c, kc in enumerate(KS):
                            nc.tensor.matmul(
                                wt_ps[:, :T],
                                v1_sb[b][:kc, c, :],
                                pT_sb[c][:kc, h * T : h * T + T],
                                start=(c == 0), stop=(c == 2),
                            )
                        r_sb = workpool.tile([1, T], F32, tag=f"r{h}", name=f"r{h}")
                        nc.vector.reciprocal(r_sb[:], wt_ps[64:65, :T])
                        rb_sb = workpool.tile([64, T], F32, tag=f"rb{h}", name=f"rb{h}")
                        if not _SKIP_BCAST:
                            nc.gpsimd.partition_broadcast(rb_sb[:], r_sb[:], channels=64)
                        else:
                            nc.vector.memset(rb_sb[:], 1.0)
                        nc.vector.tensor_tensor(
                            wTn_sb[:, h], wt_ps[:64, :T], rb_sb[:],
                            mybir.AluOpType.mult,
                        )

                    # ---- final^T = sum_h G_h^T wTn_h
                    fin_ps = pwt.tile([65, BANK], F32, tag="wt", name="fin_ps")
                    for h in range(HS):
                        nc.tensor.matmul(
                            fin_ps[:64, :T], G_sb[:, h, :], wTn_sb[:, h],
                            start=(h == 0), stop=(h == HS - 1),
                        )
                    nc.vector.tensor_copy(out_sb[:, b], fin_ps[:64, :T])

                nc.sync.dma_start(outd[:, :, j, :], out_sb[:])

    nc.compile()
    return nc


def _prep_core_inputs(q, k, v, core):
    b0 = BPC * core
    qc = q[b0 : b0 + BPC]  # (2, 7500, 64)
    kc = k[b0 : b0 + BPC]
    vc = v[b0 : b0 + BPC]
    # (b,s,h) -> (b,h,j,t) -> (128, J, T)
    qT = np.ascontiguousarray(
        qc.reshape(BPC, J, T, H).transpose(0, 3, 1, 2).reshape(BPC * H, J, T)
    )
    kT = np.ascontiguousarray(
        kc.reshape(BPC, J, T, H).transpose(0, 3, 1, 2).reshape(BPC * H, J, T)
    )
    v1 = np.zeros((BPC, J, 128, 3, 65), dtype=np.float32)
    vr = vc.reshape(BPC, J, T, H)
    for c, kcs in enumerate(KS):
        off = KOFF[c]
        v1[:, :, :kcs, c, :64] = vr[:, :, off : off + kcs, :]
        v1[:, :, :kcs, c, 64] = 1.0
    return {"qT": qT, "kT": kT, "v1": v1}


def kernel(q, k, v, Wq, Wk, Wv, Wo, _trace=False, _tmpdir=None):
    q = np.asarray(q, dtype=np.float32)
    k = np.asarray(k, dtype=np.float32)
    v = np.asarray(v, dtype=np.float32)
    Wq = np.asarray(Wq, dtype=np.float32)
    Wk = np.asarray(Wk, dtype=np.float32)
    Wv = np.asarray(Wv, dtype=np.float32)
    Wo = np.asarray(Wo, dtype=np.float32)

    scale = DK ** (-0.5)
    A = np.stack(
        [
            (Wq[:, 64 * h : 64 * h + 64] @ Wk[:, 64 * h : 64 * h + 64].T) * scale
            for h in range(HS)
        ]
    ).astype(np.float32)
    G = np.stack(
        [Wv[:, 64 * h : 64 * h + 64] @ Wo[64 * h : 64 * h + 64, :] for h in range(HS)]
    ).astype(np.float32)
    Ad = np.ascontiguousarray(
        np.concatenate([A.transpose(1, 0, 2), A.transpose(1, 0, 2)], axis=0)
    )  # (128, HS, 64)
    Gd = np.ascontiguousarray(G.transpose(1, 0, 2))  # (64, HS, 64)

    if "nc" not in _PROG_CACHE:
        _PROG_CACHE["nc"] = build_program()
    nc = _PROG_CACHE["nc"]

    in_maps = []
    for core in range(NCORES):
        m = _prep_core_inputs(q, k, v, core)
        m["Ad"] = Ad
        m["Gd"] = Gd
        in_maps.append(m)

    res = run_bass_kernel_spmd(
        nc,
        in_maps,
        core_ids=list(range(NCORES)),
        trace=_trace,
        tmpdir=_tmpdir,
    )

    out = np.empty((B, S, H), dtype=np.float32)
    for core in range(NCORES):
        o = res.results[core]["outd"]  # (64, BPC, J, T)
        out[BPC * core : BPC * core + BPC] = (
            o.transpose(1, 2, 3, 0).reshape(BPC, S, H)
        )
    if _trace:
        return out, res
    return out
